# revision 1
# baseline (speedup 1.0000x reference)
"""BA3TGCN2 Trainium2 kernel: batch-sharded GCN gather/segment-sum + GRU gate fusion.

Math (H0 == 0 makes the R gate dead and linearizes the layers):
  out[b,n,:] = sum_p ws[p] * sigmoid(-(Ahat x_p Uz + bz)) * tanh(Ahat x_p Uh + bh)
  Uz = Wcz @ Wlz[:COUT], bz = bcz @ Wlz[:COUT] + blz   (same for h with Wch/Wlh)
  ws = softmax(attention) (second half scaled by TRAIN_OR_PREDICT=1)

Sharding: batch (16) across 8 cores -> 2 batches/core. Edges replicated.
Per-core node feature row: 256 = 2 batches x 16 periods x 8 cin, bf16.
"""

import os

import numpy as np
import ml_dtypes

import concourse.bass as bass
import concourse.bacc as bacc
from concourse._compat import get_trn_type
import concourse.mybir as mybir
import concourse.tile as tile
from concourse.bass_utils import run_bass_kernel_spmd

BF16 = ml_dtypes.bfloat16

B, N, CIN, COUT, P2 = 16, 10000, 8, 32, 16
E = 160000
NCORES = 8
BPC = B // NCORES            # 2 batches per core
FEAT = BPC * P2 * CIN        # 256 features per node row per core
NBLK = (N + 127) // 128      # 79 dst blocks
NSB = (NBLK + 3) // 4        # 20 superblocks of 512 dst
CHUNKS_PER_CALL = 16         # 2048-edge gather calls
GCALL = 128 * CHUNKS_PER_CALL
TRAIN_OR_PREDICT = 1.0

LAST_RESULT = None           # BassKernelResults of last run (for test.py)


def _softmax(x):
    e = np.exp(x - np.max(x))
    return e / e.sum()


def prep_host(X, edge_index, edge_weight, attention,
              Wcz, bcz, Wlz, blz, Wcr, bcr, Wlr, blr, Wch, bch, Wlh, blh):
    """All host-side preprocessing. Returns per-core in_maps pieces + structure."""
    X = np.asarray(X, np.float32)
    src = np.asarray(edge_index[0], np.int64)
    dst = np.asarray(edge_index[1], np.int64)
    w = np.asarray(edge_weight, np.float32)

    # gcn_norm with self loops
    loop = np.arange(N, dtype=np.int64)
    src = np.concatenate([src, loop])
    dst = np.concatenate([dst, loop])
    w = np.concatenate([w, np.ones(N, np.float32)])
    deg = np.bincount(dst, weights=w, minlength=N).astype(np.float32)
    dinv = np.where(deg > 0, deg.astype(np.float64) ** -0.5, 0.0).astype(np.float32)
    norm = dinv[src] * w * dinv[dst]

    # sort by dst
    order = np.argsort(dst, kind="stable")
    src, dst, norm = src[order], dst[order], norm[order]

    # pad each 128-dst block's edge list to a multiple of 128
    blk = dst // 128
    cnt = np.bincount(blk, minlength=NBLK).astype(np.int64)
    ccnt = ((cnt + 127) // 128) * 128          # padded per-block edge counts
    nchunks_blk = (ccnt // 128).astype(np.int64)
    # pad total chunk count to a multiple of CHUNKS_PER_CALL (extra chunks on last block)
    NC = int(nchunks_blk.sum())
    pad_chunks = (-NC) % CHUNKS_PER_CALL
    nchunks_blk[-1] += pad_chunks
    ccnt[-1] += 128 * pad_chunks
    NC += pad_chunks
    EPAD = int(ccnt.sum())

    srcp = np.zeros(EPAD, np.int16)
    dstrelp = np.zeros(EPAD, np.float32)
    normp = np.zeros(EPAD, np.float32)
    out_off = np.concatenate([[0], np.cumsum(ccnt)])[:-1]
    in_off = np.concatenate([[0], np.cumsum(cnt)])[:-1]
    for k in range(NBLK):
        o, i, c = out_off[k], in_off[k], cnt[k]
        srcp[o:o + c] = src[i:i + c].astype(np.int16)
        dstrelp[o:o + c] = (dst[i:i + c] - 128 * k).astype(np.float32)
        normp[o:o + c] = norm[i:i + c]

    # gather index stream: chunk c's edge p at (p, c), int32 for indirect DMA
    gidx = np.ascontiguousarray(srcp.reshape(NC, 128).T).astype(np.int32)  # (128, NC)
    dstrel_t = np.ascontiguousarray(dstrelp.reshape(NC, 128).T)      # (128, NC) f32
    norm_t = np.ascontiguousarray(normp.reshape(NC, 128).T)          # (128, NC) f32

    # fused weights / biases / period weights
    Uz = (np.asarray(Wcz, np.float32) @ np.asarray(Wlz, np.float32)[:COUT])
    Uh = (np.asarray(Wch, np.float32) @ np.asarray(Wlh, np.float32)[:COUT])
    bz = np.asarray(bcz, np.float32) @ np.asarray(Wlz, np.float32)[:COUT] + np.asarray(blz, np.float32)
    bh = np.asarray(bch, np.float32) @ np.asarray(Wlh, np.float32)[:COUT] + np.asarray(blh, np.float32)
    probs = _softmax(np.asarray(attention, np.float32))
    ws = np.concatenate([probs[:P2 // 2], probs[P2 // 2:] * TRAIN_OR_PREDICT])

    # transform lhsT tiles: ubig[(p*8+cin), (g*4+grp)*128 + pl*32 + s] = (p==grp*4+pl)*U_g[cin,s]
    ubig = np.zeros((128, 2 * 4 * 128), np.float32)
    for g, U in enumerate((Uz, Uh)):
        for grp in range(4):
            for pl in range(4):
                p = grp * 4 + pl
                ubig[p * 8:(p + 1) * 8, (g * 4 + grp) * 128 + pl * 32:(g * 4 + grp) * 128 + (pl + 1) * 32] = U
    # weighted period-sum lhsT: wsum[(pl*32+s), grp*32+o] = ws[grp*4+pl]*(s==o)
    wsum = np.zeros((128, 4 * 32), np.float32)
    for grp in range(4):
        for pl in range(4):
            for s in range(32):
                wsum[pl * 32 + s, grp * 32 + s] = ws[grp * 4 + pl]
    biasz = np.repeat(-bz[None, :], 4, 0).reshape(128, 1).astype(np.float32)
    biash = np.repeat(bh[None, :], 4, 0).reshape(128, 1).astype(np.float32)

    iota = np.tile(np.arange(128, dtype=np.float32), (128, 1))
    ident = np.eye(128, dtype=np.float32)

    # per-core X tables: (N, 256) bf16, row layout [b(2) x p(16) x cin(8)]
    xtabs = []
    for c in range(NCORES):
        xc = np.ascontiguousarray(
            X[2 * c:2 * c + 2].transpose(1, 0, 3, 2).reshape(N, FEAT)).astype(BF16)
        xtabs.append(xc)

    shared = dict(
        gidx=gidx,
        dstrel=dstrel_t.astype(np.float32),
        normt=norm_t.astype(np.float32),
        ubig=ubig.astype(BF16),
        wsum=wsum.astype(BF16),
        biasz=biasz,
        biash=biash,
        iota=iota.astype(BF16),
        ident=ident.astype(BF16),
    )
    struct = dict(NC=NC, nchunks_blk=nchunks_blk.tolist())
    return xtabs, shared, struct


def build_bass(struct):
    NC = struct["NC"]
    nchunks_blk = struct["nchunks_blk"]

    f32 = mybir.dt.float32
    bf16 = mybir.dt.bfloat16
    i32 = mybir.dt.int32
    Alu = mybir.AluOpType
    Act = mybir.ActivationFunctionType

    nc = bacc.Bacc(get_trn_type() or "TRN2")
    xtab_d = nc.dram_tensor("xtab", (N, FEAT), bf16, kind="ExternalInput")
    gidx_d = nc.dram_tensor("gidx", (128, NC), i32, kind="ExternalInput")
    dstrel_d = nc.dram_tensor("dstrel", (128, NC), f32, kind="ExternalInput")
    normt_d = nc.dram_tensor("normt", (128, NC), f32, kind="ExternalInput")
    ubig_d = nc.dram_tensor("ubig", (128, 1024), bf16, kind="ExternalInput")
    wsum_d = nc.dram_tensor("wsum", (128, 128), bf16, kind="ExternalInput")
    biasz_d = nc.dram_tensor("biasz", (128, 1), f32, kind="ExternalInput")
    biash_d = nc.dram_tensor("biash", (128, 1), f32, kind="ExternalInput")
    iota_d = nc.dram_tensor("iota", (128, 128), bf16, kind="ExternalInput")
    ident_d = nc.dram_tensor("ident", (128, 128), bf16, kind="ExternalInput")
    out_d = nc.dram_tensor("out", (BPC, 32, N), f32, kind="ExternalOutput")

    with tile.TileContext(nc) as tc:
        with tc.tile_pool(name="const", bufs=1) as cpool, \
             tc.tile_pool(name="gp", bufs=8) as gpool, \
             tc.tile_pool(name="sp", bufs=4) as spool, \
             tc.tile_pool(name="wk", bufs=2) as wpool, \
             tc.tile_pool(name="st", bufs=1) as stpool, \
             tc.tile_pool(name="ps", bufs=1, space="PSUM") as ppool:

            def cload(dram, shape, dtype, name):
                t = cpool.tile(shape, dtype, name=name, tag=name)
                nc.sync.dma_start(t[:], dram[:])
                return t

            gidx_sb = cload(gidx_d, [128, NC], i32, "gidx_sb")
            dstrel_sb = cload(dstrel_d, [128, NC], f32, "dstrel_sb")
            norm_sb = cload(normt_d, [128, NC], f32, "norm_sb")
            ubig_sb = cload(ubig_d, [128, 1024], bf16, "ubig_sb")
            wsum_sb = cload(wsum_d, [128, 128], bf16, "wsum_sb")
            biasz_sb = cload(biasz_d, [128, 1], f32, "biasz_sb")
            biash_sb = cload(biash_d, [128, 1], f32, "biash_sb")
            iota_sb = cload(iota_d, [128, 128], bf16, "iota_sb")
            ident_sb = cload(ident_d, [128, 128], bf16, "ident_sb")

            stage = [stpool.tile([32, NSB * 512], f32, name=f"stage{b}", tag=f"stage{b}") for b in range(BPC)]

            def gather_chunk(c):
                gt = gpool.tile([128, FEAT], bf16, tag="g", name="gt")
                nc.gpsimd.indirect_dma_start(
                    out=gt[:],
                    out_offset=None,
                    in_=xtab_d[:, :],
                    in_offset=bass.IndirectOffsetOnAxis(ap=gidx_sb[:, c:c + 1], axis=0),
                )
                return gt

            chunk_base = np.concatenate([[0], np.cumsum(nchunks_blk)])
            for sb in range(NSB):
                ytA = [wpool.tile([128, 512], bf16, name=f"ytA{b}", tag=f"ytA{b}") for b in range(BPC)]
                for kb in range(4):
                    k = sb * 4 + kb
                    if k >= NBLK:
                        for b in range(BPC):
                            nc.vector.memset(ytA[b][:, kb * 128:(kb + 1) * 128], 0.0)
                        continue
                    ytb = ppool.tile([128, FEAT], f32, tag="ytb")
                    ncb = nchunks_blk[k]
                    for j in range(ncb):
                        c = int(chunk_base[k]) + j
                        gt = gather_chunk(c)
                        S = spool.tile([128, 128], bf16, tag="S")
                        nc.vector.tensor_scalar(
                            S[:], iota_sb[:],
                            dstrel_sb[:, c:c + 1], norm_sb[:, c:c + 1],
                            Alu.is_equal, Alu.mult,
                        )
                        nc.tensor.matmul(
                            ytb[:], lhsT=S[:], rhs=gt[:],
                            start=(j == 0), stop=(j == ncb - 1),
                        )
                    ysb = wpool.tile([128, FEAT], bf16, tag="ysb")
                    nc.vector.tensor_copy(ysb[:], ytb[:])
                    for b in range(BPC):
                        tp = ppool.tile([128, 128], bf16, tag="tp")
                        nc.tensor.transpose(tp[:], ysb[:, b * 128:(b + 1) * 128], ident_sb[:])
                        nc.vector.tensor_copy(ytA[b][:, kb * 128:(kb + 1) * 128], tp[:])

                for b in range(BPC):
                    ccs = []
                    for pair in range(2):
                        az = ppool.tile([128, 1024], f32, tag="az")
                        ah = ppool.tile([128, 1024], f32, tag="ah")
                        for gl in range(2):
                            grp = pair * 2 + gl
                            nc.tensor.matmul(
                                az[:, gl * 512:(gl + 1) * 512],
                                lhsT=ubig_sb[:, grp * 128:(grp + 1) * 128],
                                rhs=ytA[b][:], start=True, stop=True)
                            nc.tensor.matmul(
                                ah[:, gl * 512:(gl + 1) * 512],
                                lhsT=ubig_sb[:, (4 + grp) * 128:(5 + grp) * 128],
                                rhs=ytA[b][:], start=True, stop=True)
                        zp = wpool.tile([128, 1024], bf16, tag="zp")
                        tp2 = wpool.tile([128, 1024], bf16, tag="tp2")
                        nc.scalar.activation(zp[:], az[:], Act.Sigmoid,
                                             bias=biasz_sb[:, :1], scale=-1.0)
                        nc.scalar.activation(tp2[:], ah[:], Act.Tanh,
                                             bias=biash_sb[:, :1], scale=1.0)
                        cc = wpool.tile([128, 1024], bf16, tag="cc")
                        nc.vector.tensor_tensor(cc[:], zp[:], tp2[:], op=Alu.mult)
                        ccs.append(cc)
                    outp = ppool.tile([32, 512], f32, tag="outp")
                    for grp in range(4):
                        nc.tensor.matmul(
                            outp[:],
                            lhsT=wsum_sb[:, grp * 32:(grp + 1) * 32],
                            rhs=ccs[grp // 2][:, (grp % 2) * 512:((grp % 2) + 1) * 512],
                            start=(grp == 0), stop=(grp == 3))
                    nc.vector.tensor_copy(stage[b][:, sb * 512:(sb + 1) * 512], outp[:])

            for b in range(BPC):
                nc.sync.dma_start(out_d[b], stage[b][:, :N])

    nc.compile()
    return nc


def kernel(**inputs):
    global LAST_RESULT
    xtabs, shared, struct = prep_host(**inputs)
    nc = build_bass(struct)
    in_maps = []
    for c in range(NCORES):
        m = dict(shared)
        m["xtab"] = xtabs[c]
        in_maps.append(m)
    res = run_bass_kernel_spmd(nc, in_maps, core_ids=list(range(NCORES)),
                               trace=os.environ.get("BASS_TRACE") == "1")
    LAST_RESULT = res
    out = np.empty((B, N, COUT), np.float32)
    for c in range(NCORES):
        r = res.results[c]["out"]  # (2, 32, N)
        out[2 * c:2 * c + 2] = r.transpose(0, 2, 1)
    return out



# revision 11
# speedup vs baseline: 4.3502x; 4.3502x over previous
"""BA3TGCN2 Trainium2 kernel, v2: dst-stripe sharding.

Math (H0 == 0 makes the R gate dead and linearizes the layers):
  out[b,n,:] = sum_p ws[p] * sigmoid(-(Ahat x_p Uz + bz)) * tanh(Ahat x_p Uh + bh)
  Uz = Wcz @ Wlz[:COUT], bz = bcz @ Wlz[:COUT] + blz   (same for h with Wch/Wlh)
  ws = softmax(attention) (second half scaled by TRAIN_OR_PREDICT=1)

Sharding: dst nodes striped across 8 cores (10 dst-blocks of 128 each);
each core gathers only its own edges but full-width rows
(16 batches x 16 periods x 8 cin = 2048 bf16 = 4KB descriptors).
Per core ~180 indirect-DMA gather calls instead of 1376 -> SWDGE
descriptor-gen drops ~8x and 4KB descriptors saturate the DMA bus.
"""

import os

import numpy as np
import ml_dtypes

import concourse.bass as bass
import concourse.bacc as bacc
from concourse._compat import get_trn_type
import concourse.mybir as mybir
import concourse.tile as tile
from concourse.bass_utils import run_bass_kernel_spmd

BF16 = ml_dtypes.bfloat16

B, N, CIN, COUT, P2 = 16, 10000, 8, 32, 16
E = 160000
NCORES = 8
FEAT = B * P2 * CIN          # 2048 features per node row (full width)
NBLK = (N + 127) // 128      # 79 dst blocks total
BLKC = (NBLK + NCORES - 1) // NCORES  # 10 dst blocks per core
TRAIN_OR_PREDICT = 1.0

LAST_RESULT = None


def _softmax(x):
    e = np.exp(x - np.max(x))
    return e / e.sum()


def prep_host(X, edge_index, edge_weight, attention,
              Wcz, bcz, Wlz, blz, Wcr, bcr, Wlr, blr, Wch, bch, Wlh, blh):
    X = np.asarray(X, np.float32)
    src = np.asarray(edge_index[0], np.int64)
    dst = np.asarray(edge_index[1], np.int64)
    w = np.asarray(edge_weight, np.float32)

    # gcn_norm with self loops
    loop = np.arange(N, dtype=np.int64)
    src = np.concatenate([src, loop])
    dst = np.concatenate([dst, loop])
    w = np.concatenate([w, np.ones(N, np.float32)])
    deg = np.bincount(dst, weights=w, minlength=N).astype(np.float32)
    dinv = np.where(deg > 0, deg.astype(np.float64) ** -0.5, 0.0).astype(np.float32)
    norm = dinv[src] * w * dinv[dst]

    order = np.argsort(dst, kind="stable")
    src, dst, norm = src[order], dst[order], norm[order]

    # per-(core, block-slot) edge lists; uniform chunk grid NCB across all
    blk = dst // 128
    cnt = np.bincount(blk, minlength=NCORES * BLKC).astype(np.int64)
    NCB = int((cnt.max() + 127) // 128)
    NC = BLKC * NCB                     # gather calls per core
    in_off = np.concatenate([[0], np.cumsum(cnt)])

    gidxs, dstrels, norms = [], [], []
    for c in range(NCORES):
        srcp = np.zeros((NC, 128), np.int32)
        dstrelp = np.zeros((NC, 128), np.float32)
        normp = np.zeros((NC, 128), np.float32)
        for kb in range(BLKC):
            k = c * BLKC + kb
            if k >= NBLK:
                continue
            i0, n_k = in_off[k], cnt[k]
            fl_s = np.zeros(NCB * 128, np.int32)
            fl_d = np.zeros(NCB * 128, np.float32)
            fl_n = np.zeros(NCB * 128, np.float32)
            fl_s[:n_k] = src[i0:i0 + n_k]
            fl_d[:n_k] = (dst[i0:i0 + n_k] - 128 * k).astype(np.float32)
            fl_n[:n_k] = norm[i0:i0 + n_k]
            srcp[kb * NCB:(kb + 1) * NCB] = fl_s.reshape(NCB, 128)
            dstrelp[kb * NCB:(kb + 1) * NCB] = fl_d.reshape(NCB, 128)
            normp[kb * NCB:(kb + 1) * NCB] = fl_n.reshape(NCB, 128)
        gidxs.append(np.ascontiguousarray(srcp.T))       # (128, NC) i32
        dstrels.append(np.ascontiguousarray(dstrelp.T))  # (128, NC) f32
        norms.append(np.ascontiguousarray(normp.T))      # (128, NC) f32

    # fused weights / biases / period weights (same as linearized reference)
    Uz = np.asarray(Wcz, np.float32) @ np.asarray(Wlz, np.float32)[:COUT]
    Uh = np.asarray(Wch, np.float32) @ np.asarray(Wlh, np.float32)[:COUT]
    bz = np.asarray(bcz, np.float32) @ np.asarray(Wlz, np.float32)[:COUT] + np.asarray(blz, np.float32)
    bh = np.asarray(bch, np.float32) @ np.asarray(Wlh, np.float32)[:COUT] + np.asarray(blh, np.float32)
    probs = _softmax(np.asarray(attention, np.float32))
    ws = np.concatenate([probs[:P2 // 2], probs[P2 // 2:] * TRAIN_OR_PREDICT])

    # transform lhsT: ubig[(p*8+cin), (g*4+grp)*128 + pl*32 + s] = (p==grp*4+pl)*U_g[cin,s]
    ubig = np.zeros((128, 2 * 4 * 128), np.float32)
    for g, U in enumerate((Uz, Uh)):
        for grp in range(4):
            for pl in range(4):
                p = grp * 4 + pl
                ubig[p * 8:(p + 1) * 8,
                     (g * 4 + grp) * 128 + pl * 32:(g * 4 + grp) * 128 + (pl + 1) * 32] = U
    # weighted period-sum lhsT: wsum[(pl*32+s), grp*32+s] = ws[grp*4+pl]
    wsum = np.zeros((128, 4 * 32), np.float32)
    for grp in range(4):
        for pl in range(4):
            for s in range(32):
                wsum[pl * 32 + s, grp * 32 + s] = ws[grp * 4 + pl]
    biasz = np.repeat(-bz[None, :], 4, 0).reshape(128, 1).astype(np.float32)
    biash = np.repeat(bh[None, :], 4, 0).reshape(128, 1).astype(np.float32)

    iota = np.tile(np.arange(128, dtype=np.float32), (128, 1))
    ident = np.eye(128, dtype=np.float32)

    # shared X table: (N, 2048) bf16, feature order [b(16) x p(16) x cin(8)]
    xtab = np.ascontiguousarray(
        X.transpose(1, 0, 3, 2).reshape(N, FEAT)).astype(BF16)

    shared = dict(
        xtab=xtab,
        ubig=ubig.astype(BF16),
        wsum=wsum.astype(BF16),
        biasz=biasz,
        biash=biash,
        iota=iota.astype(BF16),
        ident=ident.astype(BF16),
    )
    percore = [dict(gidx=gidxs[c], dstrel=dstrels[c], normt=norms[c])
               for c in range(NCORES)]
    return shared, percore, NCB


def build_bass(NCB):
    NC = BLKC * NCB
    f32 = mybir.dt.float32
    bf16 = mybir.dt.bfloat16
    i32 = mybir.dt.int32
    Alu = mybir.AluOpType
    Act = mybir.ActivationFunctionType

    nc = bacc.Bacc(get_trn_type() or "TRN2")
    xtab_d = nc.dram_tensor("xtab", (N, FEAT), bf16, kind="ExternalInput")
    gidx_d = nc.dram_tensor("gidx", (128, NC), i32, kind="ExternalInput")
    dstrel_d = nc.dram_tensor("dstrel", (128, NC), f32, kind="ExternalInput")
    normt_d = nc.dram_tensor("normt", (128, NC), f32, kind="ExternalInput")
    ubig_d = nc.dram_tensor("ubig", (128, 1024), bf16, kind="ExternalInput")
    wsum_d = nc.dram_tensor("wsum", (128, 128), bf16, kind="ExternalInput")
    biasz_d = nc.dram_tensor("biasz", (128, 1), f32, kind="ExternalInput")
    biash_d = nc.dram_tensor("biash", (128, 1), f32, kind="ExternalInput")
    iota_d = nc.dram_tensor("iota", (128, 128), bf16, kind="ExternalInput")
    ident_d = nc.dram_tensor("ident", (128, 128), bf16, kind="ExternalInput")
    # out[s, (kb*16 + b)*128 + d]: block-major, then batch, then local dst
    out_d = nc.dram_tensor("out", (32, BLKC * B * 128), f32, kind="ExternalOutput")

    with tile.TileContext(nc) as tc:
        with tc.tile_pool(name="const", bufs=1) as cpool, \
             tc.tile_pool(name="gp", bufs=6) as gpool, \
             tc.tile_pool(name="sp", bufs=4) as spool, \
             tc.tile_pool(name="wk", bufs=2) as wpool, \
             tc.tile_pool(name="st", bufs=1) as stpool, \
             tc.tile_pool(name="psY", bufs=1, space="PSUM") as ppY, \
             tc.tile_pool(name="ps", bufs=1, space="PSUM") as ppool:

            def cload(dram, shape, dtype, name):
                t = cpool.tile(shape, dtype, name=name, tag=name)
                nc.sync.dma_start(t[:], dram[:])
                return t

            gidx_sb = cload(gidx_d, [128, NC], i32, "gidx_sb")
            dstrel_sb = cload(dstrel_d, [128, NC], f32, "dstrel_sb")
            norm_sb = cload(normt_d, [128, NC], f32, "norm_sb")
            ubig_sb = cload(ubig_d, [128, 1024], bf16, "ubig_sb")
            wsum_sb = cload(wsum_d, [128, 128], bf16, "wsum_sb")
            biasz_sb = cload(biasz_d, [128, 1], f32, "biasz_sb")
            biash_sb = cload(biash_d, [128, 1], f32, "biash_sb")
            iota_sb = cload(iota_d, [128, 128], bf16, "iota_sb")
            ident_sb = cload(ident_d, [128, 128], bf16, "ident_sb")

            # stage[s, (kb*16 + b)*128 + d]
            stage = stpool.tile([32, BLKC * B * 128], f32, name="stage", tag="stage")

            for kb in range(BLKC):
                # ---- aggregate: Y[dst, feat] += sum_e norm_e X[src_e] ----
                # four bank-sized psum quarters (matmul out must fit one bank)
                Yq = [ppY.tile([128, 512], f32, tag=f"Y{q}", name=f"Y{q}")
                      for q in range(4)]
                for j in range(NCB):
                    c = kb * NCB + j
                    gt = gpool.tile([128, FEAT], bf16, tag="g", name="gt")
                    nc.gpsimd.indirect_dma_start(
                        out=gt[:],
                        out_offset=None,
                        in_=xtab_d[:, :],
                        in_offset=bass.IndirectOffsetOnAxis(
                            ap=gidx_sb[:, c:c + 1], axis=0),
                    )
                    S = spool.tile([128, 128], bf16, tag="S")
                    nc.vector.tensor_scalar(
                        S[:], iota_sb[:],
                        dstrel_sb[:, c:c + 1], norm_sb[:, c:c + 1],
                        Alu.is_equal, Alu.mult,
                    )
                    for q in range(4):
                        nc.tensor.matmul(
                            Yq[q][:], lhsT=S[:],
                            rhs=gt[:, q * 512:(q + 1) * 512],
                            start=(j == 0), stop=(j == NCB - 1))

                ysb = wpool.tile([128, FEAT], bf16, tag="ysb")
                for q in range(4):
                    nc.vector.tensor_copy(
                        ysb[:, q * 512:(q + 1) * 512], Yq[q][:])

                # ---- transpose to [feat, dst] per batch (PE) ----
                ytA = wpool.tile([128, FEAT], bf16, tag="ytA")
                for b in range(B):
                    tp = ppool.tile([128, 128], bf16, tag="tp")
                    nc.tensor.transpose(
                        tp[:], ysb[:, b * 128:(b + 1) * 128], ident_sb[:])
                    nc.vector.tensor_copy(
                        ytA[:, b * 128:(b + 1) * 128], tp[:])

                # ---- transform + gates + period sum, 4 batches at a time ----
                for bg in range(4):
                    rhs4 = ytA[:, bg * 512:(bg + 1) * 512]
                    outp = ppool.tile([32, 512], f32, tag="outp")
                    for g in range(4):
                        az = ppool.tile([128, 512], f32, tag="az")
                        ah = ppool.tile([128, 512], f32, tag="ah")
                        nc.tensor.matmul(
                            az[:], lhsT=ubig_sb[:, g * 128:(g + 1) * 128],
                            rhs=rhs4, start=True, stop=True)
                        nc.tensor.matmul(
                            ah[:], lhsT=ubig_sb[:, (4 + g) * 128:(5 + g) * 128],
                            rhs=rhs4, start=True, stop=True)
                        zp = wpool.tile([128, 512], bf16, tag="zp")
                        tp2 = wpool.tile([128, 512], bf16, tag="tp2")
                        nc.scalar.activation(zp[:], az[:], Act.Sigmoid,
                                             bias=biasz_sb[:, :1], scale=-1.0)
                        nc.scalar.activation(tp2[:], ah[:], Act.Tanh,
                                             bias=biash_sb[:, :1], scale=1.0)
                        cc = wpool.tile([128, 512], bf16, tag="cc")
                        nc.vector.tensor_tensor(cc[:], zp[:], tp2[:], op=Alu.mult)
                        nc.tensor.matmul(
                            outp[:], lhsT=wsum_sb[:, g * 32:(g + 1) * 32],
                            rhs=cc[:], start=(g == 0), stop=(g == 3))
                    col = (kb * B + bg * 4) * 128
                    nc.vector.tensor_copy(stage[:, col:col + 512], outp[:])

            nc.sync.dma_start(out_d[:], stage[:])

    nc.compile()
    return nc


def kernel(**inputs):
    global LAST_RESULT
    shared, percore, NCB = prep_host(**inputs)
    nc = build_bass(NCB)
    in_maps = []
    for c in range(NCORES):
        m = dict(shared)
        m.update(percore[c])
        in_maps.append(m)
    res = run_bass_kernel_spmd(nc, in_maps, core_ids=list(range(NCORES)),
                               trace=os.environ.get("BASS_TRACE") == "1")
    LAST_RESULT = res
    out = np.empty((B, N, COUT), np.float32)
    for c in range(NCORES):
        r = np.asarray(res.results[c]["out"])  # (32, BLKC*B*128)
        r = r.reshape(32, BLKC, B, 128)        # [s, kb, b, d]
        d0 = c * BLKC * 128
        d1 = min(d0 + BLKC * 128, N)
        if d1 <= d0:
            continue
        rr = r.transpose(2, 1, 3, 0).reshape(B, BLKC * 128, COUT)
        out[:, d0:d1, :] = rr[:, :d1 - d0]
    return out


# revision 17
# speedup vs baseline: 4.4201x; 1.0161x over previous
"""BA3TGCN2 Trainium2 kernel, v2: dst-stripe sharding.

Math (H0 == 0 makes the R gate dead and linearizes the layers):
  out[b,n,:] = sum_p ws[p] * sigmoid(-(Ahat x_p Uz + bz)) * tanh(Ahat x_p Uh + bh)
  Uz = Wcz @ Wlz[:COUT], bz = bcz @ Wlz[:COUT] + blz   (same for h with Wch/Wlh)
  ws = softmax(attention) (second half scaled by TRAIN_OR_PREDICT=1)

Sharding: dst nodes striped across 8 cores (10 dst-blocks of 128 each);
each core gathers only its own edges but full-width rows
(16 batches x 16 periods x 8 cin = 2048 bf16 = 4KB descriptors).
Per core ~180 indirect-DMA gather calls instead of 1376 -> SWDGE
descriptor-gen drops ~8x and 4KB descriptors saturate the DMA bus.
"""

import os

import numpy as np
import ml_dtypes

import concourse.bass as bass
import concourse.bacc as bacc
from concourse._compat import get_trn_type
import concourse.mybir as mybir
import concourse.tile as tile
from concourse.bass_utils import run_bass_kernel_spmd

BF16 = ml_dtypes.bfloat16

B, N, CIN, COUT, P2 = 16, 10000, 8, 32, 16
E = 160000
NCORES = 8
FEAT = B * P2 * CIN          # 2048 features per node row (full width)
NBLK = (N + 127) // 128      # 79 dst blocks total
BLKC = (NBLK + NCORES - 1) // NCORES  # 10 dst blocks per core
TRAIN_OR_PREDICT = 1.0

LAST_RESULT = None


def _softmax(x):
    e = np.exp(x - np.max(x))
    return e / e.sum()


def prep_host(X, edge_index, edge_weight, attention,
              Wcz, bcz, Wlz, blz, Wcr, bcr, Wlr, blr, Wch, bch, Wlh, blh):
    X = np.asarray(X, np.float32)
    src = np.asarray(edge_index[0], np.int64)
    dst = np.asarray(edge_index[1], np.int64)
    w = np.asarray(edge_weight, np.float32)

    # gcn_norm with self loops
    loop = np.arange(N, dtype=np.int64)
    src = np.concatenate([src, loop])
    dst = np.concatenate([dst, loop])
    w = np.concatenate([w, np.ones(N, np.float32)])
    deg = np.bincount(dst, weights=w, minlength=N).astype(np.float32)
    dinv = np.where(deg > 0, deg.astype(np.float64) ** -0.5, 0.0).astype(np.float32)
    norm = dinv[src] * w * dinv[dst]

    order = np.argsort(dst, kind="stable")
    src, dst, norm = src[order], dst[order], norm[order]

    # per-(core, block-slot) edge lists; per-slot chunk count NCB_k
    # (max over cores) so the shared program wastes minimal padding
    blk = dst // 128
    cnt = np.bincount(blk, minlength=NCORES * BLKC).astype(np.int64)
    cnt2 = cnt.reshape(NCORES, BLKC)
    NCBk = [int((cnt2[:, k].max() + 127) // 128) for k in range(BLKC)]
    cbase = np.concatenate([[0], np.cumsum(NCBk)]).astype(np.int64)
    NC = int(cbase[-1])                 # gather calls per core
    in_off = np.concatenate([[0], np.cumsum(cnt)])

    gidxs, dstrels, norms = [], [], []
    for c in range(NCORES):
        srcp = np.zeros((NC, 128), np.int32)
        dstrelp = np.zeros((NC, 128), np.float32)
        normp = np.zeros((NC, 128), np.float32)
        for kb in range(BLKC):
            k = c * BLKC + kb
            ncb = NCBk[kb]
            if k >= NBLK:
                continue
            i0, n_k = in_off[k], cnt[k]
            fl_s = np.zeros(ncb * 128, np.int32)
            fl_d = np.zeros(ncb * 128, np.float32)
            fl_n = np.zeros(ncb * 128, np.float32)
            fl_s[:n_k] = src[i0:i0 + n_k]
            fl_d[:n_k] = (dst[i0:i0 + n_k] - 128 * k).astype(np.float32)
            fl_n[:n_k] = norm[i0:i0 + n_k]
            c0 = cbase[kb]
            srcp[c0:c0 + ncb] = fl_s.reshape(ncb, 128)
            dstrelp[c0:c0 + ncb] = fl_d.reshape(ncb, 128)
            normp[c0:c0 + ncb] = fl_n.reshape(ncb, 128)
        gidxs.append(np.ascontiguousarray(srcp.T))       # (128, NC) i32
        dstrels.append(np.ascontiguousarray(dstrelp.T))  # (128, NC) f32
        norms.append(np.ascontiguousarray(normp.T))      # (128, NC) f32

    # fused weights / biases / period weights (same as linearized reference)
    Uz = np.asarray(Wcz, np.float32) @ np.asarray(Wlz, np.float32)[:COUT]
    Uh = np.asarray(Wch, np.float32) @ np.asarray(Wlh, np.float32)[:COUT]
    bz = np.asarray(bcz, np.float32) @ np.asarray(Wlz, np.float32)[:COUT] + np.asarray(blz, np.float32)
    bh = np.asarray(bch, np.float32) @ np.asarray(Wlh, np.float32)[:COUT] + np.asarray(blh, np.float32)
    probs = _softmax(np.asarray(attention, np.float32))
    ws = np.concatenate([probs[:P2 // 2], probs[P2 // 2:] * TRAIN_OR_PREDICT])

    # transform lhsT: ubig[(p*8+cin), (g*4+grp)*128 + pl*32 + s] = (p==grp*4+pl)*U_g[cin,s]
    ubig = np.zeros((128, 2 * 4 * 128), np.float32)
    for g, U in enumerate((Uz, Uh)):
        for grp in range(4):
            for pl in range(4):
                p = grp * 4 + pl
                ubig[p * 8:(p + 1) * 8,
                     (g * 4 + grp) * 128 + pl * 32:(g * 4 + grp) * 128 + (pl + 1) * 32] = U
    # weighted period-sum lhsT: wsum[(pl*32+s), grp*32+s] = ws[grp*4+pl]
    wsum = np.zeros((128, 4 * 32), np.float32)
    for grp in range(4):
        for pl in range(4):
            for s in range(32):
                wsum[pl * 32 + s, grp * 32 + s] = ws[grp * 4 + pl]
    biasz = np.repeat(-bz[None, :], 4, 0).reshape(128, 1).astype(np.float32)
    biash = np.repeat(bh[None, :], 4, 0).reshape(128, 1).astype(np.float32)

    iota = np.tile(np.arange(128, dtype=np.float32), (128, 1))
    ident = np.eye(128, dtype=np.float32)

    # shared X table: (N, 2048) bf16, feature order [b(16) x p(16) x cin(8)]
    xtab = np.ascontiguousarray(
        X.transpose(1, 0, 3, 2).reshape(N, FEAT)).astype(BF16)

    shared = dict(
        xtab=xtab,
        ubig=ubig.astype(BF16),
        wsum=wsum.astype(BF16),
        biasz=biasz,
        biash=biash,
        iota=iota.astype(BF16),
        ident=ident.astype(BF16),
    )
    percore = [dict(gidx=gidxs[c], dstrel=dstrels[c], normt=norms[c])
               for c in range(NCORES)]
    return shared, percore, NCBk


def build_bass(NCBk):
    cbase = [0]
    for v in NCBk:
        cbase.append(cbase[-1] + v)
    NC = cbase[-1]
    f32 = mybir.dt.float32
    bf16 = mybir.dt.bfloat16
    f8 = mybir.dt.float8e4
    i32 = mybir.dt.int32
    Alu = mybir.AluOpType
    Act = mybir.ActivationFunctionType

    nc = bacc.Bacc(get_trn_type() or "TRN2", num_swdge_queues=2)
    xtab_d = nc.dram_tensor("xtab", (N, FEAT), bf16, kind="ExternalInput")
    gidx_d = nc.dram_tensor("gidx", (128, NC), i32, kind="ExternalInput")
    dstrel_d = nc.dram_tensor("dstrel", (128, NC), f32, kind="ExternalInput")
    normt_d = nc.dram_tensor("normt", (128, NC), f32, kind="ExternalInput")
    ubig_d = nc.dram_tensor("ubig", (128, 1024), bf16, kind="ExternalInput")
    wsum_d = nc.dram_tensor("wsum", (128, 128), bf16, kind="ExternalInput")
    biasz_d = nc.dram_tensor("biasz", (128, 1), f32, kind="ExternalInput")
    biash_d = nc.dram_tensor("biash", (128, 1), f32, kind="ExternalInput")
    iota_d = nc.dram_tensor("iota", (128, 128), bf16, kind="ExternalInput")
    ident_d = nc.dram_tensor("ident", (128, 128), bf16, kind="ExternalInput")
    # out[s, (kb*16 + b)*128 + d]: block-major, then batch, then local dst
    out_d = nc.dram_tensor("out", (32, BLKC * B * 128), f32, kind="ExternalOutput")

    with tile.TileContext(nc) as tc:
        with tc.tile_pool(name="const", bufs=1) as cpool, \
             tc.tile_pool(name="gp", bufs=6) as gpool, \
             tc.tile_pool(name="sp", bufs=4) as spool, \
             tc.tile_pool(name="wk", bufs=2) as wpool, \
             tc.tile_pool(name="st", bufs=1) as stpool, \
             tc.tile_pool(name="psY", bufs=1, space="PSUM") as ppY, \
             tc.tile_pool(name="ps", bufs=1, space="PSUM") as ppool:

            def cload(dram, shape, dtype, name):
                t = cpool.tile(shape, dtype, name=name, tag=name)
                nc.sync.dma_start(t[:], dram[:])
                return t

            gidx_sb = cload(gidx_d, [128, NC], i32, "gidx_sb")
            dstrel_sb = cload(dstrel_d, [128, NC], f32, "dstrel_sb")
            norm_sb = cload(normt_d, [128, NC], f32, "norm_sb")
            ubig_sb = cload(ubig_d, [128, 1024], bf16, "ubig_sb")
            wsum_sb = cload(wsum_d, [128, 128], bf16, "wsum_sb")
            biasz_sb = cload(biasz_d, [128, 1], f32, "biasz_sb")
            biash_sb = cload(biash_d, [128, 1], f32, "biash_sb")
            iota_sb = cload(iota_d, [128, 128], bf16, "iota_sb")
            ident_sb = cload(ident_d, [128, 128], bf16, "ident_sb")

            # stage[s, (kb*16 + b)*128 + d]
            stage = stpool.tile([32, BLKC * B * 128], f32, name="stage", tag="stage")

            def transform_units(kb, ysb):
                """Yield the transform of block kb as schedulable units."""
                ytA = wpool.tile([128, FEAT], bf16, tag="ytA")

                def tp_quad(q):
                    def run():
                        for r in range(4):
                            b = q * 4 + r
                            tp = ppool.tile([128, 128], bf16, tag="tp")
                            nc.tensor.transpose(
                                tp[:], ysb[:, b * 128:(b + 1) * 128],
                                ident_sb[:])
                            nc.vector.tensor_copy(
                                ytA[:, b * 128:(b + 1) * 128], tp[:])
                    return run

                def bg_unit(bg):
                    def run():
                        rhs4 = ytA[:, bg * 512:(bg + 1) * 512]
                        outp = ppool.tile([32, 512], f32, tag="outp")
                        for g in range(4):
                            az = ppool.tile([128, 512], f32, tag="az")
                            ah = ppool.tile([128, 512], f32, tag="ah")
                            nc.tensor.matmul(
                                az[:], lhsT=ubig_sb[:, g * 128:(g + 1) * 128],
                                rhs=rhs4, start=True, stop=True)
                            nc.tensor.matmul(
                                ah[:],
                                lhsT=ubig_sb[:, (4 + g) * 128:(5 + g) * 128],
                                rhs=rhs4, start=True, stop=True)
                            zp = wpool.tile([128, 512], bf16, tag="zp")
                            tp2 = wpool.tile([128, 512], bf16, tag="tp2")
                            nc.scalar.activation(zp[:], az[:], Act.Sigmoid,
                                                 bias=biasz_sb[:, :1],
                                                 scale=-1.0)
                            nc.scalar.activation(tp2[:], ah[:], Act.Tanh,
                                                 bias=biash_sb[:, :1],
                                                 scale=1.0)
                            cc = wpool.tile([128, 512], bf16, tag="cc")
                            nc.vector.tensor_tensor(cc[:], zp[:], tp2[:],
                                                    op=Alu.mult)
                            nc.tensor.matmul(
                                outp[:], lhsT=wsum_sb[:, g * 32:(g + 1) * 32],
                                rhs=cc[:], start=(g == 0), stop=(g == 3))
                        col = (kb * B + bg * 4) * 128
                        nc.vector.tensor_copy(stage[:, col:col + 512], outp[:])
                    return run

                return [tp_quad(q) for q in range(4)] + \
                       [bg_unit(bg) for bg in range(4)]

            # software pipeline: interleave transform(kb-1) units into the
            # aggregation loop of block kb so the PE stream never drains
            pending = []
            for kb in range(BLKC):
                Yq = [ppY.tile([128, 512], f32, tag=f"Y{q}", name=f"Y{q}")
                      for q in range(4)]
                ncb = NCBk[kb]
                stride = max(1, ncb // (len(pending) + 1)) if pending else ncb + 1
                u = 0
                for j in range(ncb):
                    c = cbase[kb] + j
                    gt = gpool.tile([128, FEAT], bf16, tag="g", name="gt")
                    di = nc.gpsimd.indirect_dma_start(
                        out=gt[:],
                        out_offset=None,
                        in_=xtab_d[:, :],
                        in_offset=bass.IndirectOffsetOnAxis(
                            ap=gidx_sb[:, c:c + 1], axis=0),
                    )
                    if c % 2:
                        di.ins.queue = "qPoolDynamic1"
                    S = spool.tile([128, 128], bf16, tag="S")
                    nc.vector.tensor_scalar(
                        S[:], iota_sb[:],
                        dstrel_sb[:, c:c + 1], norm_sb[:, c:c + 1],
                        Alu.is_equal, Alu.mult,
                    )
                    for q in range(4):
                        nc.tensor.matmul(
                            Yq[q][:], lhsT=S[:],
                            rhs=gt[:, q * 512:(q + 1) * 512],
                            start=(j == 0), stop=(j == ncb - 1))
                    if j % stride == stride - 1 and u < len(pending):
                        pending[u]()
                        u += 1
                while u < len(pending):
                    pending[u]()
                    u += 1

                ysb = wpool.tile([128, FEAT], bf16, tag="ysb")
                for q in range(4):
                    nc.vector.tensor_copy(
                        ysb[:, q * 512:(q + 1) * 512], Yq[q][:])
                pending = transform_units(kb, ysb)

            for unit in pending:
                unit()

            nc.sync.dma_start(out_d[:], stage[:])

    nc.compile()
    return nc


def kernel(**inputs):
    global LAST_RESULT
    shared, percore, NCBk = prep_host(**inputs)
    nc = build_bass(NCBk)
    in_maps = []
    for c in range(NCORES):
        m = dict(shared)
        m.update(percore[c])
        in_maps.append(m)
    res = run_bass_kernel_spmd(nc, in_maps, core_ids=list(range(NCORES)),
                               trace=os.environ.get("BASS_TRACE") == "1")
    LAST_RESULT = res
    out = np.empty((B, N, COUT), np.float32)
    for c in range(NCORES):
        r = np.asarray(res.results[c]["out"])  # (32, BLKC*B*128)
        r = r.reshape(32, BLKC, B, 128)        # [s, kb, b, d]
        d0 = c * BLKC * 128
        d1 = min(d0 + BLKC * 128, N)
        if d1 <= d0:
            continue
        rr = r.transpose(2, 1, 3, 0).reshape(B, BLKC * 128, COUT)
        out[:, d0:d1, :] = rr[:, :d1 - d0]
    return out


# revision 19
# speedup vs baseline: 5.1897x; 1.1741x over previous
"""BA3TGCN2 Trainium2 kernel, v2: dst-stripe sharding.

Math (H0 == 0 makes the R gate dead and linearizes the layers):
  out[b,n,:] = sum_p ws[p] * sigmoid(-(Ahat x_p Uz + bz)) * tanh(Ahat x_p Uh + bh)
  Uz = Wcz @ Wlz[:COUT], bz = bcz @ Wlz[:COUT] + blz   (same for h with Wch/Wlh)
  ws = softmax(attention) (second half scaled by TRAIN_OR_PREDICT=1)

Sharding: dst nodes striped across 8 cores (10 dst-blocks of 128 each);
each core gathers only its own edges but full-width rows
(16 batches x 16 periods x 8 cin = 2048 bf16 = 4KB descriptors).
Per core ~180 indirect-DMA gather calls instead of 1376 -> SWDGE
descriptor-gen drops ~8x and 4KB descriptors saturate the DMA bus.
"""

import os

import numpy as np
import ml_dtypes

import concourse.bass as bass
import concourse.bacc as bacc
from concourse._compat import get_trn_type
import concourse.mybir as mybir
import concourse.tile as tile
from concourse.bass_utils import run_bass_kernel_spmd

BF16 = ml_dtypes.bfloat16

B, N, CIN, COUT, P2 = 16, 10000, 8, 32, 16
E = 160000
NCORES = 8
FEAT = B * P2 * CIN          # 2048 features per node row (full width)
NBLK = (N + 127) // 128      # 79 dst blocks total
BLKC = (NBLK + NCORES - 1) // NCORES  # 10 dst blocks per core
TRAIN_OR_PREDICT = 1.0

LAST_RESULT = None


def _softmax(x):
    e = np.exp(x - np.max(x))
    return e / e.sum()


def prep_host(X, edge_index, edge_weight, attention,
              Wcz, bcz, Wlz, blz, Wcr, bcr, Wlr, blr, Wch, bch, Wlh, blh):
    X = np.asarray(X, np.float32)
    src = np.asarray(edge_index[0], np.int64)
    dst = np.asarray(edge_index[1], np.int64)
    w = np.asarray(edge_weight, np.float32)

    # gcn_norm with self loops
    loop = np.arange(N, dtype=np.int64)
    src = np.concatenate([src, loop])
    dst = np.concatenate([dst, loop])
    w = np.concatenate([w, np.ones(N, np.float32)])
    deg = np.bincount(dst, weights=w, minlength=N).astype(np.float32)
    dinv = np.where(deg > 0, deg.astype(np.float64) ** -0.5, 0.0).astype(np.float32)
    norm = dinv[src] * w * dinv[dst]

    order = np.argsort(dst, kind="stable")
    src, dst, norm = src[order], dst[order], norm[order]

    # per-(core, block-slot) edge lists; per-slot chunk count NCB_k
    # (max over cores) so the shared program wastes minimal padding
    blk = dst // 128
    cnt = np.bincount(blk, minlength=NCORES * BLKC).astype(np.int64)
    cnt2 = cnt.reshape(NCORES, BLKC)
    NCBk = [int((cnt2[:, k].max() + 127) // 128) for k in range(BLKC)]
    cbase = np.concatenate([[0], np.cumsum(NCBk)]).astype(np.int64)
    NC = int(cbase[-1])                 # gather calls per core
    in_off = np.concatenate([[0], np.cumsum(cnt)])

    gidxs, dstrels, norms = [], [], []
    for c in range(NCORES):
        srcp = np.zeros((NC, 128), np.int32)
        dstrelp = np.zeros((NC, 128), np.float32)
        normp = np.zeros((NC, 128), np.float32)
        for kb in range(BLKC):
            k = c * BLKC + kb
            ncb = NCBk[kb]
            if k >= NBLK:
                continue
            i0, n_k = in_off[k], cnt[k]
            # sort the block's edges by src so gather descriptors walk
            # ascending HBM addresses (row-buffer locality)
            so = np.argsort(src[i0:i0 + n_k], kind="stable")
            fl_s = np.zeros(ncb * 128, np.int32)
            fl_d = np.zeros(ncb * 128, np.float32)
            fl_n = np.zeros(ncb * 128, np.float32)
            fl_s[:n_k] = src[i0:i0 + n_k][so]
            fl_d[:n_k] = (dst[i0:i0 + n_k][so] - 128 * k).astype(np.float32)
            fl_n[:n_k] = norm[i0:i0 + n_k][so]
            c0 = cbase[kb]
            srcp[c0:c0 + ncb] = fl_s.reshape(ncb, 128)
            dstrelp[c0:c0 + ncb] = fl_d.reshape(ncb, 128)
            normp[c0:c0 + ncb] = fl_n.reshape(ncb, 128)
        gidxs.append(np.ascontiguousarray(srcp.T))       # (128, NC) i32
        dstrels.append(np.ascontiguousarray(dstrelp.T))  # (128, NC) f32
        norms.append(np.ascontiguousarray(normp.T))      # (128, NC) f32

    # fused weights / biases / period weights (same as linearized reference)
    Uz = np.asarray(Wcz, np.float32) @ np.asarray(Wlz, np.float32)[:COUT]
    Uh = np.asarray(Wch, np.float32) @ np.asarray(Wlh, np.float32)[:COUT]
    bz = np.asarray(bcz, np.float32) @ np.asarray(Wlz, np.float32)[:COUT] + np.asarray(blz, np.float32)
    bh = np.asarray(bch, np.float32) @ np.asarray(Wlh, np.float32)[:COUT] + np.asarray(blh, np.float32)
    probs = _softmax(np.asarray(attention, np.float32))
    ws = np.concatenate([probs[:P2 // 2], probs[P2 // 2:] * TRAIN_OR_PREDICT])

    # transform lhsT: ubig[(p*8+cin), (g*4+grp)*128 + pl*32 + s] = (p==grp*4+pl)*U_g[cin,s]
    ubig = np.zeros((128, 2 * 4 * 128), np.float32)
    for g, U in enumerate((Uz, Uh)):
        for grp in range(4):
            for pl in range(4):
                p = grp * 4 + pl
                ubig[p * 8:(p + 1) * 8,
                     (g * 4 + grp) * 128 + pl * 32:(g * 4 + grp) * 128 + (pl + 1) * 32] = U
    # weighted period-sum lhsT: wsum[(pl*32+s), grp*32+s] = ws[grp*4+pl]
    wsum = np.zeros((128, 4 * 32), np.float32)
    for grp in range(4):
        for pl in range(4):
            for s in range(32):
                wsum[pl * 32 + s, grp * 32 + s] = ws[grp * 4 + pl]
    biasz = np.repeat(-bz[None, :], 4, 0).reshape(128, 1).astype(np.float32)
    biash = np.repeat(bh[None, :], 4, 0).reshape(128, 1).astype(np.float32)

    iota = np.tile(np.arange(128, dtype=np.float32), (128, 1))
    ident = np.eye(128, dtype=np.float32)

    # shared X table: (N, 2048) bf16, feature order [b(16) x p(16) x cin(8)]
    xtab = np.ascontiguousarray(
        X.transpose(1, 0, 3, 2).reshape(N, FEAT)).astype(BF16)

    shared = dict(
        xtab=xtab,
        ubig=ubig.astype(BF16),
        wsum=wsum.astype(BF16),
        biasz=biasz,
        biash=biash,
        iota=iota.astype(BF16),
        ident=ident.astype(BF16),
    )
    percore = [dict(gidx=gidxs[c], dstrel=dstrels[c], normt=norms[c])
               for c in range(NCORES)]
    return shared, percore, NCBk


def build_bass(NCBk):
    cbase = [0]
    for v in NCBk:
        cbase.append(cbase[-1] + v)
    NC = cbase[-1]
    f32 = mybir.dt.float32
    bf16 = mybir.dt.bfloat16
    f8 = mybir.dt.float8e4
    i32 = mybir.dt.int32
    Alu = mybir.AluOpType
    Act = mybir.ActivationFunctionType

    nc = bacc.Bacc(get_trn_type() or "TRN2", num_swdge_queues=2)
    xtab_d = nc.dram_tensor("xtab", (N, FEAT), bf16, kind="ExternalInput")
    gidx_d = nc.dram_tensor("gidx", (128, NC), i32, kind="ExternalInput")
    dstrel_d = nc.dram_tensor("dstrel", (128, NC), f32, kind="ExternalInput")
    normt_d = nc.dram_tensor("normt", (128, NC), f32, kind="ExternalInput")
    ubig_d = nc.dram_tensor("ubig", (128, 1024), bf16, kind="ExternalInput")
    wsum_d = nc.dram_tensor("wsum", (128, 128), bf16, kind="ExternalInput")
    biasz_d = nc.dram_tensor("biasz", (128, 1), f32, kind="ExternalInput")
    biash_d = nc.dram_tensor("biash", (128, 1), f32, kind="ExternalInput")
    iota_d = nc.dram_tensor("iota", (128, 128), bf16, kind="ExternalInput")
    ident_d = nc.dram_tensor("ident", (128, 128), bf16, kind="ExternalInput")
    # out[s, (kb*16 + b)*128 + d]: block-major, then batch, then local dst
    out_d = nc.dram_tensor("out", (32, BLKC * B * 128), f32, kind="ExternalOutput")

    with tile.TileContext(nc) as tc:
        with tc.tile_pool(name="const", bufs=1) as cpool, \
             tc.tile_pool(name="gp", bufs=10) as gpool, \
             tc.tile_pool(name="sp", bufs=8) as spool, \
             tc.tile_pool(name="wk", bufs=2) as wpool, \
             tc.tile_pool(name="st", bufs=1) as stpool, \
             tc.tile_pool(name="psY", bufs=1, space="PSUM") as ppY, \
             tc.tile_pool(name="ps", bufs=1, space="PSUM") as ppool:

            def cload(dram, shape, dtype, name):
                t = cpool.tile(shape, dtype, name=name, tag=name)
                nc.sync.dma_start(t[:], dram[:])
                return t

            gidx_sb = cload(gidx_d, [128, NC], i32, "gidx_sb")
            dstrel_sb = cload(dstrel_d, [128, NC], f32, "dstrel_sb")
            norm_sb = cload(normt_d, [128, NC], f32, "norm_sb")
            ubig_sb = cload(ubig_d, [128, 1024], bf16, "ubig_sb")
            wsum_sb = cload(wsum_d, [128, 128], bf16, "wsum_sb")
            biasz_sb = cload(biasz_d, [128, 1], f32, "biasz_sb")
            biash_sb = cload(biash_d, [128, 1], f32, "biash_sb")
            iota_sb = cload(iota_d, [128, 128], bf16, "iota_sb")
            ident_sb = cload(ident_d, [128, 128], bf16, "ident_sb")

            # stage[s, (kb*16 + b)*128 + d]
            stage = stpool.tile([32, BLKC * B * 128], f32, name="stage", tag="stage")

            def transform_units(kb, ysb):
                """Yield the transform of block kb as schedulable units."""
                ytA = wpool.tile([128, FEAT], bf16, tag="ytA")

                def tp_quad(q):
                    def run():
                        for r in range(4):
                            b = q * 4 + r
                            tp = ppool.tile([128, 128], bf16, tag="tp")
                            nc.tensor.transpose(
                                tp[:], ysb[:, b * 128:(b + 1) * 128],
                                ident_sb[:])
                            nc.vector.tensor_copy(
                                ytA[:, b * 128:(b + 1) * 128], tp[:])
                    return run

                def bg_unit(bg):
                    def run():
                        rhs4 = ytA[:, bg * 512:(bg + 1) * 512]
                        outp = ppool.tile([32, 512], f32, tag="outp")
                        for g in range(4):
                            az = ppool.tile([128, 512], f32, tag="az")
                            ah = ppool.tile([128, 512], f32, tag="ah")
                            nc.tensor.matmul(
                                az[:], lhsT=ubig_sb[:, g * 128:(g + 1) * 128],
                                rhs=rhs4, start=True, stop=True)
                            nc.tensor.matmul(
                                ah[:],
                                lhsT=ubig_sb[:, (4 + g) * 128:(5 + g) * 128],
                                rhs=rhs4, start=True, stop=True)
                            zp = wpool.tile([128, 512], bf16, tag="zp")
                            tp2 = wpool.tile([128, 512], bf16, tag="tp2")
                            nc.scalar.activation(zp[:], az[:], Act.Sigmoid,
                                                 bias=biasz_sb[:, :1],
                                                 scale=-1.0)
                            nc.scalar.activation(tp2[:], ah[:], Act.Tanh,
                                                 bias=biash_sb[:, :1],
                                                 scale=1.0)
                            cc = wpool.tile([128, 512], bf16, tag="cc")
                            nc.vector.tensor_tensor(cc[:], zp[:], tp2[:],
                                                    op=Alu.mult)
                            nc.tensor.matmul(
                                outp[:], lhsT=wsum_sb[:, g * 32:(g + 1) * 32],
                                rhs=cc[:], start=(g == 0), stop=(g == 3))
                        col = (kb * B + bg * 4) * 128
                        nc.vector.tensor_copy(stage[:, col:col + 512], outp[:])
                    return run

                return [tp_quad(q) for q in range(4)] + \
                       [bg_unit(bg) for bg in range(4)]

            # software pipeline: interleave transform(kb-1) units into the
            # aggregation loop of block kb so the PE stream never drains
            pending = []
            for kb in range(BLKC):
                Yq = [ppY.tile([128, 512], f32, tag=f"Y{q}", name=f"Y{q}")
                      for q in range(4)]
                ncb = NCBk[kb]
                stride = max(1, ncb // (len(pending) + 1)) if pending else ncb + 1
                u = 0
                for j in range(ncb):
                    c = cbase[kb] + j
                    gt = gpool.tile([128, FEAT], bf16, tag="g", name="gt")
                    di = nc.gpsimd.indirect_dma_start(
                        out=gt[:],
                        out_offset=None,
                        in_=xtab_d[:, :],
                        in_offset=bass.IndirectOffsetOnAxis(
                            ap=gidx_sb[:, c:c + 1], axis=0),
                    )
                    if c % 2:
                        di.ins.queue = "qPoolDynamic1"
                    S = spool.tile([128, 128], bf16, tag="S")
                    nc.vector.tensor_scalar(
                        S[:], iota_sb[:],
                        dstrel_sb[:, c:c + 1], norm_sb[:, c:c + 1],
                        Alu.is_equal, Alu.mult,
                    )
                    for q in range(4):
                        nc.tensor.matmul(
                            Yq[q][:], lhsT=S[:],
                            rhs=gt[:, q * 512:(q + 1) * 512],
                            start=(j == 0), stop=(j == ncb - 1))
                    if j % stride == stride - 1 and u < len(pending):
                        pending[u]()
                        u += 1
                while u < len(pending):
                    pending[u]()
                    u += 1

                ysb = wpool.tile([128, FEAT], bf16, tag="ysb")
                for q in range(4):
                    nc.vector.tensor_copy(
                        ysb[:, q * 512:(q + 1) * 512], Yq[q][:])
                pending = transform_units(kb, ysb)

            for unit in pending:
                unit()

            nc.sync.dma_start(out_d[:], stage[:])

    nc.compile()
    return nc


def kernel(**inputs):
    global LAST_RESULT
    shared, percore, NCBk = prep_host(**inputs)
    nc = build_bass(NCBk)
    in_maps = []
    for c in range(NCORES):
        m = dict(shared)
        m.update(percore[c])
        in_maps.append(m)
    res = run_bass_kernel_spmd(nc, in_maps, core_ids=list(range(NCORES)),
                               trace=os.environ.get("BASS_TRACE") == "1")
    LAST_RESULT = res
    out = np.empty((B, N, COUT), np.float32)
    for c in range(NCORES):
        r = np.asarray(res.results[c]["out"])  # (32, BLKC*B*128)
        r = r.reshape(32, BLKC, B, 128)        # [s, kb, b, d]
        d0 = c * BLKC * 128
        d1 = min(d0 + BLKC * 128, N)
        if d1 <= d0:
            continue
        rr = r.transpose(2, 1, 3, 0).reshape(B, BLKC * 128, COUT)
        out[:, d0:d1, :] = rr[:, :d1 - d0]
    return out


# revision 21
# speedup vs baseline: 5.3521x; 1.0313x over previous
"""BA3TGCN2 Trainium2 kernel, v2: dst-stripe sharding.

Math (H0 == 0 makes the R gate dead and linearizes the layers):
  out[b,n,:] = sum_p ws[p] * sigmoid(-(Ahat x_p Uz + bz)) * tanh(Ahat x_p Uh + bh)
  Uz = Wcz @ Wlz[:COUT], bz = bcz @ Wlz[:COUT] + blz   (same for h with Wch/Wlh)
  ws = softmax(attention) (second half scaled by TRAIN_OR_PREDICT=1)

Sharding: dst nodes striped across 8 cores (10 dst-blocks of 128 each);
each core gathers only its own edges but full-width rows
(16 batches x 16 periods x 8 cin = 2048 bf16 = 4KB descriptors).
Per core ~180 indirect-DMA gather calls instead of 1376 -> SWDGE
descriptor-gen drops ~8x and 4KB descriptors saturate the DMA bus.
"""

import os

import numpy as np
import ml_dtypes

import concourse.bass as bass
import concourse.bacc as bacc
from concourse._compat import get_trn_type
import concourse.mybir as mybir
import concourse.tile as tile
from concourse.bass_utils import run_bass_kernel_spmd

BF16 = ml_dtypes.bfloat16

B, N, CIN, COUT, P2 = 16, 10000, 8, 32, 16
E = 160000
NCORES = 8
FEAT = B * P2 * CIN          # 2048 features per node row (full width)
NBLK = (N + 127) // 128      # 79 dst blocks total
BLKC = (NBLK + NCORES - 1) // NCORES  # 10 dst blocks per core
TRAIN_OR_PREDICT = 1.0

LAST_RESULT = None


def _softmax(x):
    e = np.exp(x - np.max(x))
    return e / e.sum()


def prep_host(X, edge_index, edge_weight, attention,
              Wcz, bcz, Wlz, blz, Wcr, bcr, Wlr, blr, Wch, bch, Wlh, blh):
    X = np.asarray(X, np.float32)
    src = np.asarray(edge_index[0], np.int64)
    dst = np.asarray(edge_index[1], np.int64)
    w = np.asarray(edge_weight, np.float32)

    # gcn_norm with self loops
    loop = np.arange(N, dtype=np.int64)
    src = np.concatenate([src, loop])
    dst = np.concatenate([dst, loop])
    w = np.concatenate([w, np.ones(N, np.float32)])
    deg = np.bincount(dst, weights=w, minlength=N).astype(np.float32)
    dinv = np.where(deg > 0, deg.astype(np.float64) ** -0.5, 0.0).astype(np.float32)
    norm = dinv[src] * w * dinv[dst]

    order = np.argsort(dst, kind="stable")
    src, dst, norm = src[order], dst[order], norm[order]

    # Balance dst blocks across (core, slot): rank blocks by edge count and
    # give slot s the 8 blocks of rank [8s, 8s+8) -> per-slot max (NCB_k)
    # stays close to the mean and program padding is minimal.
    blk = dst // 128
    cnt = np.bincount(blk, minlength=NBLK).astype(np.int64)
    cnt80 = np.concatenate([cnt, np.zeros(NCORES * BLKC - NBLK, np.int64)])
    order = np.argsort(-cnt80, kind="stable")
    bmap = np.empty((NCORES, BLKC), np.int64)
    for s in range(BLKC):
        for c in range(NCORES):
            bmap[c, s] = order[s * NCORES + c]
    NCBk = [int((cnt80[bmap[:, s]].max() + 127) // 128) for s in range(BLKC)]
    NCBk = [max(v, 1) for v in NCBk]
    cbase = np.concatenate([[0], np.cumsum(NCBk)]).astype(np.int64)
    NC = int(cbase[-1])                 # gather calls per core
    in_off = np.concatenate([[0], np.cumsum(cnt)])

    gidxs, dstrels, norms = [], [], []
    for c in range(NCORES):
        srcp = np.zeros((NC, 128), np.int32)
        dstrelp = np.zeros((NC, 128), np.float32)
        normp = np.zeros((NC, 128), np.float32)
        for kb in range(BLKC):
            k = int(bmap[c, kb])
            ncb = NCBk[kb]
            if k >= NBLK:
                continue
            i0, n_k = in_off[k], cnt[k]
            # sort the block's edges by src so gather descriptors walk
            # ascending HBM addresses (row-buffer locality)
            so = np.argsort(src[i0:i0 + n_k], kind="stable")
            fl_s = np.zeros(ncb * 128, np.int32)
            fl_d = np.zeros(ncb * 128, np.float32)
            fl_n = np.zeros(ncb * 128, np.float32)
            fl_s[:n_k] = src[i0:i0 + n_k][so]
            fl_d[:n_k] = (dst[i0:i0 + n_k][so] - 128 * k).astype(np.float32)
            fl_n[:n_k] = norm[i0:i0 + n_k][so]
            c0 = cbase[kb]
            srcp[c0:c0 + ncb] = fl_s.reshape(ncb, 128)
            dstrelp[c0:c0 + ncb] = fl_d.reshape(ncb, 128)
            normp[c0:c0 + ncb] = fl_n.reshape(ncb, 128)
        gidxs.append(np.ascontiguousarray(srcp.T))       # (128, NC) i32
        dstrels.append(np.ascontiguousarray(dstrelp.T))  # (128, NC) f32
        norms.append(np.ascontiguousarray(normp.T))      # (128, NC) f32

    # fused weights / biases / period weights (same as linearized reference)
    Uz = np.asarray(Wcz, np.float32) @ np.asarray(Wlz, np.float32)[:COUT]
    Uh = np.asarray(Wch, np.float32) @ np.asarray(Wlh, np.float32)[:COUT]
    bz = np.asarray(bcz, np.float32) @ np.asarray(Wlz, np.float32)[:COUT] + np.asarray(blz, np.float32)
    bh = np.asarray(bch, np.float32) @ np.asarray(Wlh, np.float32)[:COUT] + np.asarray(blh, np.float32)
    probs = _softmax(np.asarray(attention, np.float32))
    ws = np.concatenate([probs[:P2 // 2], probs[P2 // 2:] * TRAIN_OR_PREDICT])

    # transform lhsT: ubig[(p*8+cin), (g*4+grp)*128 + pl*32 + s] = (p==grp*4+pl)*U_g[cin,s]
    ubig = np.zeros((128, 2 * 4 * 128), np.float32)
    for g, U in enumerate((Uz, Uh)):
        for grp in range(4):
            for pl in range(4):
                p = grp * 4 + pl
                ubig[p * 8:(p + 1) * 8,
                     (g * 4 + grp) * 128 + pl * 32:(g * 4 + grp) * 128 + (pl + 1) * 32] = U
    # weighted period-sum lhsT: wsum[(pl*32+s), grp*32+s] = ws[grp*4+pl]
    wsum = np.zeros((128, 4 * 32), np.float32)
    for grp in range(4):
        for pl in range(4):
            for s in range(32):
                wsum[pl * 32 + s, grp * 32 + s] = ws[grp * 4 + pl]
    biasz = np.repeat(-bz[None, :], 4, 0).reshape(128, 1).astype(np.float32)
    biash = np.repeat(bh[None, :], 4, 0).reshape(128, 1).astype(np.float32)

    iota = np.tile(np.arange(128, dtype=np.float32), (128, 1))
    ident = np.eye(128, dtype=np.float32)

    # shared X table: (N, 2048) bf16, feature order [b(16) x p(16) x cin(8)]
    xtab = np.ascontiguousarray(
        X.transpose(1, 0, 3, 2).reshape(N, FEAT)).astype(BF16)

    shared = dict(
        xtab=xtab,
        ubig=ubig.astype(BF16),
        wsum=wsum.astype(BF16),
        biasz=biasz,
        biash=biash,
        iota=iota.astype(BF16),
        ident=ident.astype(BF16),
    )
    percore = [dict(gidx=gidxs[c], dstrel=dstrels[c], normt=norms[c])
               for c in range(NCORES)]
    return shared, percore, NCBk, bmap


def build_bass(NCBk):
    cbase = [0]
    for v in NCBk:
        cbase.append(cbase[-1] + v)
    NC = cbase[-1]
    f32 = mybir.dt.float32
    bf16 = mybir.dt.bfloat16
    f8 = mybir.dt.float8e4
    i32 = mybir.dt.int32
    Alu = mybir.AluOpType
    Act = mybir.ActivationFunctionType

    nc = bacc.Bacc(get_trn_type() or "TRN2", num_swdge_queues=2)
    xtab_d = nc.dram_tensor("xtab", (N, FEAT), bf16, kind="ExternalInput")
    gidx_d = nc.dram_tensor("gidx", (128, NC), i32, kind="ExternalInput")
    dstrel_d = nc.dram_tensor("dstrel", (128, NC), f32, kind="ExternalInput")
    normt_d = nc.dram_tensor("normt", (128, NC), f32, kind="ExternalInput")
    ubig_d = nc.dram_tensor("ubig", (128, 1024), bf16, kind="ExternalInput")
    wsum_d = nc.dram_tensor("wsum", (128, 128), bf16, kind="ExternalInput")
    biasz_d = nc.dram_tensor("biasz", (128, 1), f32, kind="ExternalInput")
    biash_d = nc.dram_tensor("biash", (128, 1), f32, kind="ExternalInput")
    iota_d = nc.dram_tensor("iota", (128, 128), bf16, kind="ExternalInput")
    ident_d = nc.dram_tensor("ident", (128, 128), bf16, kind="ExternalInput")
    # out[s, (kb*16 + b)*128 + d]: block-major, then batch, then local dst
    out_d = nc.dram_tensor("out", (32, BLKC * B * 128), f32, kind="ExternalOutput")

    with tile.TileContext(nc) as tc:
        with tc.tile_pool(name="const", bufs=1) as cpool, \
             tc.tile_pool(name="gp", bufs=16) as gpool, \
             tc.tile_pool(name="sp", bufs=8) as spool, \
             tc.tile_pool(name="wk", bufs=2) as wpool, \
             tc.tile_pool(name="st", bufs=1) as stpool, \
             tc.tile_pool(name="psY", bufs=1, space="PSUM") as ppY, \
             tc.tile_pool(name="ps", bufs=1, space="PSUM") as ppool:

            def cload(dram, shape, dtype, name):
                t = cpool.tile(shape, dtype, name=name, tag=name)
                nc.sync.dma_start(t[:], dram[:])
                return t

            gidx_sb = cload(gidx_d, [128, NC], i32, "gidx_sb")
            dstrel_sb = cload(dstrel_d, [128, NC], f32, "dstrel_sb")
            norm_sb = cload(normt_d, [128, NC], f32, "norm_sb")
            ubig_sb = cload(ubig_d, [128, 1024], bf16, "ubig_sb")
            wsum_sb = cload(wsum_d, [128, 128], bf16, "wsum_sb")
            biasz_sb = cload(biasz_d, [128, 1], f32, "biasz_sb")
            biash_sb = cload(biash_d, [128, 1], f32, "biash_sb")
            iota_sb = cload(iota_d, [128, 128], bf16, "iota_sb")
            ident_sb = cload(ident_d, [128, 128], bf16, "ident_sb")

            # stage[s, (kb*16 + b)*128 + d]
            stage = stpool.tile([32, BLKC * B * 128], f32, name="stage", tag="stage")

            def transform_units(kb, ysb):
                """Yield the transform of block kb as schedulable units."""
                ytA = wpool.tile([128, FEAT], bf16, tag="ytA")

                def tp_quad(q):
                    def run():
                        for r in range(4):
                            b = q * 4 + r
                            tp = ppool.tile([128, 128], bf16, tag="tp")
                            nc.tensor.transpose(
                                tp[:], ysb[:, b * 128:(b + 1) * 128],
                                ident_sb[:])
                            nc.vector.tensor_copy(
                                ytA[:, b * 128:(b + 1) * 128], tp[:])
                    return run

                def bg_unit(bg):
                    def run():
                        rhs4 = ytA[:, bg * 512:(bg + 1) * 512]
                        outp = ppool.tile([32, 512], f32, tag="outp")
                        for g in range(4):
                            az = ppool.tile([128, 512], f32, tag="az")
                            ah = ppool.tile([128, 512], f32, tag="ah")
                            nc.tensor.matmul(
                                az[:], lhsT=ubig_sb[:, g * 128:(g + 1) * 128],
                                rhs=rhs4, start=True, stop=True)
                            nc.tensor.matmul(
                                ah[:],
                                lhsT=ubig_sb[:, (4 + g) * 128:(5 + g) * 128],
                                rhs=rhs4, start=True, stop=True)
                            zp = wpool.tile([128, 512], bf16, tag="zp")
                            tp2 = wpool.tile([128, 512], bf16, tag="tp2")
                            nc.scalar.activation(zp[:], az[:], Act.Sigmoid,
                                                 bias=biasz_sb[:, :1],
                                                 scale=-1.0)
                            nc.scalar.activation(tp2[:], ah[:], Act.Tanh,
                                                 bias=biash_sb[:, :1],
                                                 scale=1.0)
                            cc = wpool.tile([128, 512], bf16, tag="cc")
                            nc.vector.tensor_tensor(cc[:], zp[:], tp2[:],
                                                    op=Alu.mult)
                            nc.tensor.matmul(
                                outp[:], lhsT=wsum_sb[:, g * 32:(g + 1) * 32],
                                rhs=cc[:], start=(g == 0), stop=(g == 3))
                        col = (kb * B + bg * 4) * 128
                        nc.vector.tensor_copy(stage[:, col:col + 512], outp[:])
                    return run

                return [tp_quad(q) for q in range(4)] + \
                       [bg_unit(bg) for bg in range(4)]

            # software pipeline: interleave transform(kb-1) units into the
            # aggregation loop of block kb so the PE stream never drains
            pending = []
            for kb in range(BLKC):
                Yq = [ppY.tile([128, 512], f32, tag=f"Y{q}", name=f"Y{q}")
                      for q in range(4)]
                ncb = NCBk[kb]
                stride = max(1, ncb // (len(pending) + 1)) if pending else ncb + 1
                u = 0
                for j in range(ncb):
                    c = cbase[kb] + j
                    gt = gpool.tile([128, FEAT], bf16, tag="g", name="gt")
                    di = nc.gpsimd.indirect_dma_start(
                        out=gt[:],
                        out_offset=None,
                        in_=xtab_d[:, :],
                        in_offset=bass.IndirectOffsetOnAxis(
                            ap=gidx_sb[:, c:c + 1], axis=0),
                    )
                    if c % 2:
                        di.ins.queue = "qPoolDynamic1"
                    S = spool.tile([128, 128], bf16, tag="S")
                    nc.vector.tensor_scalar(
                        S[:], iota_sb[:],
                        dstrel_sb[:, c:c + 1], norm_sb[:, c:c + 1],
                        Alu.is_equal, Alu.mult,
                    )
                    for q in range(4):
                        nc.tensor.matmul(
                            Yq[q][:], lhsT=S[:],
                            rhs=gt[:, q * 512:(q + 1) * 512],
                            start=(j == 0), stop=(j == ncb - 1))
                    if j % stride == stride - 1 and u < len(pending):
                        pending[u]()
                        u += 1
                while u < len(pending):
                    pending[u]()
                    u += 1

                ysb = wpool.tile([128, FEAT], bf16, tag="ysb")
                for q in range(4):
                    nc.vector.tensor_copy(
                        ysb[:, q * 512:(q + 1) * 512], Yq[q][:])
                pending = transform_units(kb, ysb)

            for unit in pending:
                unit()

            nc.sync.dma_start(out_d[:], stage[:])

    nc.compile()
    return nc


def kernel(**inputs):
    global LAST_RESULT
    shared, percore, NCBk, bmap = prep_host(**inputs)
    nc = build_bass(NCBk)
    in_maps = []
    for c in range(NCORES):
        m = dict(shared)
        m.update(percore[c])
        in_maps.append(m)
    res = run_bass_kernel_spmd(nc, in_maps, core_ids=list(range(NCORES)),
                               trace=os.environ.get("BASS_TRACE") == "1")
    LAST_RESULT = res
    out = np.empty((B, N, COUT), np.float32)
    for c in range(NCORES):
        r = np.asarray(res.results[c]["out"])  # (32, BLKC*B*128)
        r = r.reshape(32, BLKC, B, 128)        # [s, slot, b, d]
        for s in range(BLKC):
            k = int(bmap[c, s])
            if k >= NBLK:
                continue
            d0 = k * 128
            d1 = min(d0 + 128, N)
            out[:, d0:d1, :] = r[:, s, :, :d1 - d0].transpose(1, 2, 0)
    return out
